# revision 12
# baseline (speedup 1.0000x reference)
"""Vanilla RNN (h_t = tanh(x_t @ Wx + h_{t-1} @ Wh + b)) on 8 trn2 cores.

Sharding: data-parallel over batch (64 -> 8 per core); weights replicated.

Per-core design:
  - hidden state kept TRANSPOSED in SBUF: stage[p=h%128, col=(t_loc, c, b)]
    (c = h-chunk of 128, b = batch 8).  tanh output in this layout feeds the
    next step's matmul rhs directly (no per-step transpose in the scan chain).
  - scan step: psum[:, c*8:+8] += Wh[kc][c].T @ hT_prev[kc]  (4 small mms),
    one DVE add of the precomputed xp slice, one fused tanh (both h-chunks).
  - xp = x @ Wx + b precomputed per 64-step block: x loaded naturally
    (t on partitions), PE-transposed to get d on partitions, then 2 big mms.
    Interleaved one block ahead of the scan so all of it hides under the
    sequential recurrence.
  - output: stage blocks PE-transposed back to (t,b)-rows x h-cols, stored
    with h-contiguous DMA runs.
"""

import numpy as np

import concourse.bacc as bacc
import concourse.mybir as mybir
from concourse.tile import TileContext
from concourse.bass_utils import run_bass_kernel_spmd
from concourse.masks import make_identity

B, T, D, H = 64, 1024, 256, 256
NCORES = 8
BL = B // NCORES  # 8 batch per core
P = 128
NBLK = T // 64  # 16 blocks of 64 steps
F32 = mybir.dt.float32

_nc_cache = {}


def build_nc():
    nc = bacc.Bacc()
    x = nc.declare_dram_parameter("x", [BL, T, D], F32, isOutput=False)
    h0t = nc.declare_dram_parameter("h0t", [2, P, BL], F32, isOutput=False)
    Wx = nc.declare_dram_parameter("Wx", [D, H], F32, isOutput=False)
    Wh = nc.declare_dram_parameter("Wh", [H, H], F32, isOutput=False)
    b2 = nc.declare_dram_parameter("b2", [P, 2], F32, isOutput=False)
    hs = nc.declare_dram_parameter("hs", [BL, T, H], F32, isOutput=True)

    with TileContext(nc) as tc:
        with (
            tc.tile_pool(name="const", bufs=1) as const,
            tc.tile_pool(name="xnat", bufs=3) as xnp_,
            tc.tile_pool(name="xt", bufs=4) as xtp,
            tc.tile_pool(name="xp", bufs=3) as xpp,
            tc.tile_pool(name="stage", bufs=3) as stp,
            tc.tile_pool(name="ostage", bufs=8) as osp,
            tc.tile_pool(name="ps_scan", bufs=4, space="PSUM") as pss,
            tc.tile_pool(name="ps_p1", bufs=2, space="PSUM") as psp,
        ):
            # ---- constants ----
            wh = [[const.tile([P, P], F32, name=f"wh{k}{m}", tag=f"wh{k}{m}")
                   for m in range(2)] for k in range(2)]
            wx = [[const.tile([P, P], F32, name=f"wx{k}{m}", tag=f"wx{k}{m}")
                   for m in range(2)] for k in range(2)]
            for k in range(2):
                for m in range(2):
                    nc.sync.dma_start(
                        out=wh[k][m][:],
                        in_=Wh[k * P:(k + 1) * P, m * P:(m + 1) * P])
                    nc.sync.dma_start(
                        out=wx[k][m][:],
                        in_=Wx[k * P:(k + 1) * P, m * P:(m + 1) * P])
            bias = const.tile([P, 2], F32, tag="bias")
            nc.sync.dma_start(out=bias[:], in_=b2[:])
            h_init = const.tile([P, 16], F32, tag="h_init")
            for c in range(2):
                nc.sync.dma_start(out=h_init[:, c * 8:(c + 1) * 8],
                                  in_=h0t[c])
            ident = const.tile([P, P], F32, tag="ident")
            make_identity(nc, ident[:])

            xp_blks = [None] * NBLK
            stage_blks = [None] * NBLK

            def phase1(nt):
                """Compute xpT for block nt: xp_blk[p, tl*16 + c*8 + b]."""
                # natural x load: partitions = t (64), cols = (b, d)
                xnat = xnp_.tile([64, BL * D], F32, tag="xnat")
                nc.sync.dma_start(
                    out=xnat[:].rearrange("p (b d) -> p b d", d=D),
                    in_=x[:, nt * 64:(nt + 1) * 64, :].rearrange(
                        "b t d -> t b d"))
                # PE-transpose to xt[kc][p=d, col=(t,b)]
                xts = [xtp.tile([P, 64 * BL], F32, name=f"xt{kc}",
                                tag=f"xt{kc}") for kc in range(2)]
                for bi in range(BL):
                    for kc in range(2):
                        pst = psp.tile([P, P], F32, tag="ps_tr")
                        nc.tensor.transpose(
                            pst[:, 0:64],
                            xnat[:, bi * D + kc * P:bi * D + (kc + 1) * P],
                            ident[0:64, 0:64])
                        nc.vector.tensor_copy(
                            xts[kc][:].rearrange("p (t b) -> p t b",
                                                 b=BL)[:, :, bi],
                            pst[:, 0:64])
                blk = xpp.tile([P, 64 * 16], F32, tag="xp_blk")
                xp_blks[nt] = blk
                for mc in range(2):
                    ps = psp.tile([P, 64 * BL], F32, tag="ps_p1")
                    nc.tensor.matmul(ps[:], wx[0][mc], xts[0][:],
                                     start=True, stop=False)
                    nc.tensor.matmul(ps[:], wx[1][mc], xts[1][:],
                                     start=False, stop=True)
                    # psum -> sbuf with per-partition bias add
                    # xp_blk col layout: c*512 + t*8 + b  (c-major)
                    nc.vector.tensor_scalar_add(
                        blk[:, mc * 512:(mc + 1) * 512],
                        ps[:],
                        bias[:, mc:mc + 1])

            def scan_block(nt):
                # stage col layout: c*512 + t*8 + b  (c-major, so 2D slices
                # for both the scan rhs and the store-transpose lhsT)
                stage = stp.tile([P, 64 * 16], F32, tag="stage")
                stage_blks[nt] = stage
                blk = xp_blks[nt]
                for tl in range(64):
                    t = nt * 64 + tl
                    if t == 0:
                        prev, ptl = h_init, 0
                        pstride = 8  # h_init layout: c*8 + b
                    elif tl == 0:
                        prev, ptl = stage_blks[nt - 1], 63
                        pstride = 512
                    else:
                        prev, ptl = stage, tl - 1
                        pstride = 512
                    toff = ptl * 8 if pstride == 512 else 0
                    ps = pss.tile([P, 16], F32, tag="ps_scan")
                    for mc in range(2):
                        o = ps[:, mc * 8:(mc + 1) * 8]
                        for kc in range(2):
                            base = kc * pstride + toff
                            nc.tensor.matmul(o, wh[kc][mc],
                                             prev[:, base:base + 8],
                                             start=(kc == 0), stop=(kc == 1))
                    nc.vector.tensor_add(
                        ps[:].rearrange("p (c b) -> p c b", c=2),
                        ps[:].rearrange("p (c b) -> p c b", c=2),
                        blk[:].rearrange("p (c tb) -> p c tb",
                                         c=2)[:, :, tl * 8:(tl + 1) * 8])
                    nc.scalar.activation(
                        stage[:].rearrange("p (c tb) -> p c tb",
                                           c=2)[:, :, tl * 8:(tl + 1) * 8],
                        ps[:].rearrange("p (c b) -> p c b", c=2),
                        mybir.ActivationFunctionType.Tanh)

            def store_block(nt):
                """PE-transpose stage back to rows=(t,b), cols=h; DMA out."""
                stage = stage_blks[nt]
                for cc in range(4):  # 16 t-steps per chunk
                    ost = osp.tile([P, H], F32, tag="ostage")
                    for c in range(2):
                        pst = psp.tile([P, P], F32, tag="ps_tr")
                        nc.tensor.transpose(
                            pst[:],
                            stage[:, c * 512 + cc * P:c * 512 + (cc + 1) * P],
                            ident[:])
                        nc.vector.tensor_copy(ost[:, c * P:(c + 1) * P],
                                              pst[:])
                    for bi in range(BL):
                        nc.sync.dma_start(
                            out=hs[bi, nt * 64 + cc * 16:
                                   nt * 64 + (cc + 1) * 16, :],
                            in_=ost[bi::BL, :])

            phase1(0)
            for nt in range(NBLK):
                if nt + 1 < NBLK:
                    phase1(nt + 1)
                scan_block(nt)
                store_block(nt)

    nc.finalize()
    return nc


def kernel(x, h0, Wx, Wh, b, _trace=False, _tmpdir=None):
    if "nc" not in _nc_cache:
        _nc_cache["nc"] = build_nc()
    nc = _nc_cache["nc"]

    x = np.ascontiguousarray(np.asarray(x, dtype=np.float32))
    h0 = np.ascontiguousarray(np.asarray(h0, dtype=np.float32))
    Wx = np.ascontiguousarray(np.asarray(Wx, dtype=np.float32))
    Wh = np.ascontiguousarray(np.asarray(Wh, dtype=np.float32))
    b = np.ascontiguousarray(np.asarray(b, dtype=np.float32))
    b2 = np.ascontiguousarray(b.reshape(2, P).T)  # [128, 2]

    in_maps = []
    for c in range(NCORES):
        h0s = h0[c * BL:(c + 1) * BL]  # [8, 256]
        h0t = np.ascontiguousarray(
            h0s.reshape(BL, 2, P).transpose(1, 2, 0))  # [2, 128, 8]
        in_maps.append({
            "x": x[c * BL:(c + 1) * BL],
            "h0t": h0t,
            "Wx": Wx, "Wh": Wh, "b2": b2,
        })
    res = run_bass_kernel_spmd(nc, in_maps, core_ids=list(range(NCORES)),
                               trace=_trace, tmpdir=_tmpdir)
    hs = np.concatenate([res.results[c]["hs"] for c in range(NCORES)], axis=0)
    kernel._last_exec_time_ns = res.exec_time_ns
    return hs, hs[:, -1, :].copy()


# revision 14
# speedup vs baseline: 1.4815x; 1.4815x over previous
"""Vanilla RNN (h_t = tanh(x_t @ Wx + h_{t-1} @ Wh + b)) on 8 trn2 cores.

Sharding: data-parallel over batch (64 -> 8 per core); weights replicated.

Per-core design:
  - hidden state kept TRANSPOSED in SBUF: stage[p=h%128, col=(t_loc, c, b)]
    (c = h-chunk of 128, b = batch 8).  tanh output in this layout feeds the
    next step's matmul rhs directly (no per-step transpose in the scan chain).
  - scan step: psum[:, c*8:+8] += Wh[kc][c].T @ hT_prev[kc]  (4 small mms),
    one DVE add of the precomputed xp slice, one fused tanh (both h-chunks).
  - xp = x @ Wx + b precomputed per 64-step block: x loaded naturally
    (t on partitions), PE-transposed to get d on partitions, then 2 big mms.
    Interleaved one block ahead of the scan so all of it hides under the
    sequential recurrence.
  - output: stage blocks PE-transposed back to (t,b)-rows x h-cols, stored
    with h-contiguous DMA runs.
"""

import numpy as np

import concourse.bacc as bacc
import concourse.mybir as mybir
from concourse.tile import TileContext
from concourse.bass_utils import run_bass_kernel_spmd
from concourse.masks import make_identity

B, T, D, H = 64, 1024, 256, 256
NCORES = 8
BL = B // NCORES  # 8 batch per core
P = 128
NBLK = T // 64  # 16 blocks of 64 steps
F32 = mybir.dt.float32
F32R = mybir.dt.float32r
MM_DT = F32R

_nc_cache = {}


def build_nc():
    nc = bacc.Bacc()
    x = nc.declare_dram_parameter("x", [BL, T, D], F32, isOutput=False)
    h0t = nc.declare_dram_parameter("h0t", [2, P, BL], F32, isOutput=False)
    Wx = nc.declare_dram_parameter("Wx", [D, H], F32, isOutput=False)
    Wh = nc.declare_dram_parameter("Wh", [H, H], F32, isOutput=False)
    b2 = nc.declare_dram_parameter("b2", [P, 2], F32, isOutput=False)
    hs = nc.declare_dram_parameter("hs", [BL, T, H], F32, isOutput=True)

    with TileContext(nc) as tc:
        with (
            tc.tile_pool(name="const", bufs=1) as const,
            tc.tile_pool(name="xnat", bufs=3) as xnp_,
            tc.tile_pool(name="xt", bufs=4) as xtp,
            tc.tile_pool(name="xp", bufs=3) as xpp,
            tc.tile_pool(name="stage", bufs=3) as stp,
            tc.tile_pool(name="ostage", bufs=8) as osp,
            tc.tile_pool(name="ps_scan", bufs=4, space="PSUM") as pss,
            tc.tile_pool(name="ps_p1", bufs=2, space="PSUM") as psp,
        ):
            # ---- constants ----
            wh = [[const.tile([P, P], F32, name=f"wh{k}{m}", tag=f"wh{k}{m}")
                   for m in range(2)] for k in range(2)]
            wx = [[const.tile([P, P], F32, name=f"wx{k}{m}", tag=f"wx{k}{m}")
                   for m in range(2)] for k in range(2)]
            for k in range(2):
                for m in range(2):
                    nc.sync.dma_start(
                        out=wh[k][m][:],
                        in_=Wh[k * P:(k + 1) * P, m * P:(m + 1) * P])
                    nc.sync.dma_start(
                        out=wx[k][m][:],
                        in_=Wx[k * P:(k + 1) * P, m * P:(m + 1) * P])
            bias = const.tile([P, 2], F32, tag="bias")
            nc.sync.dma_start(out=bias[:], in_=b2[:])
            h_init = const.tile([P, 16], F32, tag="h_init")
            for c in range(2):
                nc.sync.dma_start(out=h_init[:, c * 8:(c + 1) * 8],
                                  in_=h0t[c])
            ident = const.tile([P, P], F32, tag="ident")
            make_identity(nc, ident[:])
            whr = [[const.tile([P, P], F32R, name=f"whr{k}{m}", tag=f"whr{k}{m}")
                    for m in range(2)] for k in range(2)]
            wxr = [[const.tile([P, P], F32R, name=f"wxr{k}{m}", tag=f"wxr{k}{m}")
                    for m in range(2)] for k in range(2)]
            for k in range(2):
                for m in range(2):
                    nc.vector.tensor_copy(whr[k][m][:], wh[k][m][:])
                    nc.vector.tensor_copy(wxr[k][m][:], wx[k][m][:])
            h_initr = const.tile([P, 16], F32R, tag="h_initr")
            nc.vector.tensor_copy(h_initr[:], h_init[:])

            xp_blks = [None] * NBLK
            stage_blks = [None] * NBLK

            def phase1(nt):
                """Compute xpT for block nt: xp_blk[p, tl*16 + c*8 + b]."""
                # natural x load: partitions = t (64), cols = (b, d)
                xnat = xnp_.tile([64, BL * D], F32, tag="xnat")
                nc.sync.dma_start(
                    out=xnat[:].rearrange("p (b d) -> p b d", d=D),
                    in_=x[:, nt * 64:(nt + 1) * 64, :].rearrange(
                        "b t d -> t b d"))
                # PE-transpose to xt[kc][p=d, col=(t,b)]
                xts = [xtp.tile([P, 64 * BL], F32R, name=f"xt{kc}",
                                tag=f"xt{kc}") for kc in range(2)]
                for bi in range(BL):
                    for kc in range(2):
                        pst = psp.tile([P, P], F32, tag="ps_tr")
                        nc.tensor.transpose(
                            pst[:, 0:64],
                            xnat[:, bi * D + kc * P:bi * D + (kc + 1) * P],
                            ident[0:64, 0:64])
                        nc.vector.tensor_copy(
                            xts[kc][:].rearrange("p (t b) -> p t b",
                                                 b=BL)[:, :, bi],
                            pst[:, 0:64])
                blk = xpp.tile([P, 64 * 16], F32, tag="xp_blk")
                xp_blks[nt] = blk
                for mc in range(2):
                    ps = psp.tile([P, 64 * BL], F32, tag="ps_p1")
                    nc.tensor.matmul(ps[:], wxr[0][mc], xts[0][:],
                                     start=True, stop=False)
                    nc.tensor.matmul(ps[:], wxr[1][mc], xts[1][:],
                                     start=False, stop=True)
                    # psum -> sbuf with per-partition bias add
                    # xp_blk col layout: c*512 + t*8 + b  (c-major)
                    nc.vector.tensor_scalar_add(
                        blk[:, mc * 512:(mc + 1) * 512],
                        ps[:],
                        bias[:, mc:mc + 1])

            def scan_block(nt):
                # stage col layout: c*512 + t*8 + b  (c-major, so 2D slices
                # for both the scan rhs and the store-transpose lhsT)
                stage = stp.tile([P, 64 * 16], F32R, tag="stage")
                stage_blks[nt] = stage
                blk = xp_blks[nt]
                for tl in range(64):
                    t = nt * 64 + tl
                    if t == 0:
                        prev, ptl = h_initr, 0
                        pstride = 8  # h_init layout: c*8 + b
                    elif tl == 0:
                        prev, ptl = stage_blks[nt - 1], 63
                        pstride = 512
                    else:
                        prev, ptl = stage, tl - 1
                        pstride = 512
                    toff = ptl * 8 if pstride == 512 else 0
                    ps = pss.tile([P, 16], F32, tag="ps_scan")
                    for mc in range(2):
                        o = ps[:, mc * 8:(mc + 1) * 8]
                        for kc in range(2):
                            base = kc * pstride + toff
                            nc.tensor.matmul(o, whr[kc][mc],
                                             prev[:, base:base + 8],
                                             start=(kc == 0), stop=(kc == 1))
                    nc.vector.tensor_add(
                        ps[:].rearrange("p (c b) -> p c b", c=2),
                        ps[:].rearrange("p (c b) -> p c b", c=2),
                        blk[:].rearrange("p (c tb) -> p c tb",
                                         c=2)[:, :, tl * 8:(tl + 1) * 8])
                    nc.scalar.activation(
                        stage[:].rearrange("p (c tb) -> p c tb",
                                           c=2)[:, :, tl * 8:(tl + 1) * 8],
                        ps[:].rearrange("p (c b) -> p c b", c=2),
                        mybir.ActivationFunctionType.Tanh)

            def store_block(nt):
                """PE-transpose stage back to rows=(t,b), cols=h; DMA out."""
                stage = stage_blks[nt]
                for cc in range(4):  # 16 t-steps per chunk
                    ost = osp.tile([P, H], F32, tag="ostage")
                    for c in range(2):
                        pst = psp.tile([P, P], F32, tag="ps_tr")
                        nc.tensor.transpose(
                            pst[:],
                            stage[:, c * 512 + cc * P:
                                  c * 512 + (cc + 1) * P].bitcast(F32),
                            ident[:])
                        nc.vector.tensor_copy(ost[:, c * P:(c + 1) * P],
                                              pst[:])
                    for bi in range(BL):
                        nc.sync.dma_start(
                            out=hs[bi, nt * 64 + cc * 16:
                                   nt * 64 + (cc + 1) * 16, :],
                            in_=ost[bi::BL, :])

            phase1(0)
            for nt in range(NBLK):
                if nt + 1 < NBLK:
                    phase1(nt + 1)
                scan_block(nt)
                store_block(nt)

    nc.finalize()
    return nc


def kernel(x, h0, Wx, Wh, b, _trace=False, _tmpdir=None):
    if "nc" not in _nc_cache:
        _nc_cache["nc"] = build_nc()
    nc = _nc_cache["nc"]

    x = np.ascontiguousarray(np.asarray(x, dtype=np.float32))
    h0 = np.ascontiguousarray(np.asarray(h0, dtype=np.float32))
    Wx = np.ascontiguousarray(np.asarray(Wx, dtype=np.float32))
    Wh = np.ascontiguousarray(np.asarray(Wh, dtype=np.float32))
    b = np.ascontiguousarray(np.asarray(b, dtype=np.float32))
    b2 = np.ascontiguousarray(b.reshape(2, P).T)  # [128, 2]

    in_maps = []
    for c in range(NCORES):
        h0s = h0[c * BL:(c + 1) * BL]  # [8, 256]
        h0t = np.ascontiguousarray(
            h0s.reshape(BL, 2, P).transpose(1, 2, 0))  # [2, 128, 8]
        in_maps.append({
            "x": x[c * BL:(c + 1) * BL],
            "h0t": h0t,
            "Wx": Wx, "Wh": Wh, "b2": b2,
        })
    res = run_bass_kernel_spmd(nc, in_maps, core_ids=list(range(NCORES)),
                               trace=_trace, tmpdir=_tmpdir)
    hs = np.concatenate([res.results[c]["hs"] for c in range(NCORES)], axis=0)
    kernel._last_exec_time_ns = res.exec_time_ns
    return hs, hs[:, -1, :].copy()
